# revision 4
# baseline (speedup 1.0000x reference)
"""Causal self-attention (B=4, T=2048, C=1024, H=16, HD=64) on 8 trn2 cores.

Sharding: core = (batch b, head-group g), g in {0,1} covering 8 heads each.
Each core: QKV projection for its 8 heads, causal attention, partial output
projection y_g @ W_proj[g*512:(g+1)*512]. Host sums the two partials + b_proj.

Step-1 kernel (vs baseline): x/weights stream in bf16 (half the DMA); the
QK projection runs fp8e4 DoubleRow (contraction 256/instr, 0.5 cyc/col) with
W pre-scaled x16 on host (keeps fp8 normals) and un-scaled in the PSUM->SBUF
copy; exp runs on kc-PAIRS as one wide ACT op for full blocks; QK bias copies
moved from ACT to DVE (scalar_tensor_tensor) to keep ACT for exp; QK weight
tiles fully resident; n-major projection sweep so attention starts early.
"""

import numpy as np

B, T, C, H, HD = 4, 2048, 1024, 16, 64
G = 2              # head groups (tensor parallel)
HG = H // G        # 8 heads per group
GC = HG * HD       # 512 group channels
P = 128
NQC = T // 512     # 4 q-chunks of 512
NKC = T // P       # 16 k-chunks of 128
KO_C = C // P      # 8 contraction chunks for C=1024
KO_G = GC // P     # 4 contraction chunks for GC=512
WSCALE = 16.0      # host-side premultiplier on W_attn qk columns (fp8 normals)

_cache = {}


def _build():
    import concourse.bass as bass
    import concourse.tile as tile
    from concourse import bacc, mybir

    f32 = mybir.dt.float32
    f32r = mybir.dt.float32r
    bf16 = mybir.dt.bfloat16
    fp8 = mybir.dt.float8e4
    DR = mybir.MatmulPerfMode.DoubleRow

    nc = bacc.Bacc(name="csa")
    x8 = nc.declare_dram_parameter("x8", [P, KO_C, T], fp8, isOutput=False)
    xbf = nc.declare_dram_parameter("xbf", [P, KO_C, T], bf16, isOutput=False)
    wqk = nc.declare_dram_parameter("wqk", [2 * GC // P, P, KO_C, P], fp8, isOutput=False)
    bqk = nc.declare_dram_parameter("bqk", [P, 2 * GC // P], f32, isOutput=False)
    wv = nc.declare_dram_parameter("wv", [P, KO_C, GC], bf16, isOutput=False)
    bv = nc.declare_dram_parameter("bv", [P, GC], f32, isOutput=False)
    wp = nc.declare_dram_parameter("wp", [P, KO_G, C], bf16, isOutput=False)
    mask = nc.declare_dram_parameter("mask", [P, P], f32r, isOutput=False)
    out = nc.declare_dram_parameter("out", [T, C], f32, isOutput=True)

    from contextlib import ExitStack

    with tile.TileContext(nc) as tc, ExitStack() as ctx:
            singles = ctx.enter_context(tc.tile_pool(name="singles", bufs=1))
            ppool = ctx.enter_context(tc.tile_pool(name="ppool", bufs=3))
            spool = ctx.enter_context(tc.tile_pool(name="spool", bufs=2))
            pp = ctx.enter_context(tc.tile_pool(name="pp", bufs=2, space="PSUM"))
            ps = ctx.enter_context(tc.tile_pool(name="ps", bufs=1, space="PSUM"))
            py = ctx.enter_context(tc.tile_pool(name="py", bufs=2, space="PSUM"))
            # ---- resident tensors ----
            xbf_s = singles.tile([P, KO_C, T], bf16, tag="xbf")   # V-proj lhsT
            x8_s = singles.tile([P, KO_C, T], fp8, tag="x8")      # QK-proj rhs
            wqk_s = singles.tile([P, 2 * GC // P, KO_C, P], fp8, tag="wqk")

            # T-major chunks so the first QK matmul can start early
            def emit_x8(_n, after=None):
                d = None
                for _ko in range(KO_C):
                    d = nc.sync.dma_start(
                        out=x8_s[:, _ko, _n * 512:(_n + 1) * 512],
                        in_=x8[:, _ko, _n * 512:(_n + 1) * 512],
                    )
                    if after is not None:
                        tile.add_dep_helper(d.ins, after.ins, reason="dma order")
                return d

            def emit_xbf(_n, after=None):
                d = None
                for _ko in range(KO_C):
                    d = nc.sync.dma_start(
                        out=xbf_s[:, _ko, _n * 512:(_n + 1) * 512],
                        in_=xbf[:, _ko, _n * 512:(_n + 1) * 512],
                    )
                    if after is not None:
                        tile.add_dep_helper(d.ins, after.ins, reason="dma order")
                return d

            d_x80 = emit_x8(0)
            for _m in range(2 * GC // P):
                d_wqk = nc.sync.dma_start(out=wqk_s[:, _m], in_=wqk[_m])
                tile.add_dep_helper(d_wqk.ins, d_x80.ins, reason="dma order")
            d_xbf0 = emit_xbf(0, after=d_x80)
            wv_s = singles.tile([P, KO_C, GC], bf16, tag="wv")
            d_wv = None
            for _ko in range(KO_C):
                d_wv = nc.sync.dma_start(out=wv_s[:, _ko, :], in_=wv[:, _ko, :])
                tile.add_dep_helper(d_wv.ins, d_xbf0.ins, reason="dma order")
            d_prev = d_wv
            for _n in range(1, NQC):
                d_prev = emit_x8(_n, after=d_prev)
                d_prev = emit_xbf(_n, after=d_prev)
            wp_s = singles.tile([P, KO_G, C], bf16, tag="wp")
            for _ko in range(KO_G):
                _d = nc.sync.dma_start(out=wp_s[:, _ko, :], in_=wp[:, _ko, :])
                tile.add_dep_helper(_d.ins, d_prev.ins, reason="dma order")

            QT = singles.tile([P, HG // 2, T], bf16, tag="QT")
            KT = singles.tile([P, HG // 2, T], bf16, tag="KT")
            # V augmented: cols 0..63 = V, col 64 = ones (softmax denominator)
            vaug = singles.tile([P, NKC, HG, 65], f32r, tag="vaug")
            ones_sb = singles.tile([P, 1], f32, tag="ones_sb")
            nc.vector.memset(ones_sb[:], 1.0)
            nc.vector.tensor_copy(
                out=vaug[:, :, :, 64:65],
                in_=ones_sb[:, :, None, None].to_broadcast((P, NKC, HG, 1)),
            )
            tri = singles.tile([P, P], f32r, tag="tri")
            nc.sync.dma_start(out=tri[:], in_=mask[:])
            bqk_s = singles.tile([P, 2 * GC // P], f32, tag="bqk")
            nc.sync.dma_start(out=bqk_s[:], in_=bqk[:])
            bv_s = singles.tile([P, HG, HD], f32, tag="bv")
            nc.sync.dma_start(out=bv_s[:], in_=bv.rearrange("p (h d) -> p h d", h=HG))

            # ---- QK projection: fp8 DoubleRow, contraction 256/instr ----
            def emit_qk_group(m, n):
                acc = pp.tile([P, 512], f32, tag="pp")
                for k2 in range(KO_C // 2):
                    nc.tensor.matmul(
                        acc[:],
                        lhsT=wqk_s[:, m, 2 * k2:2 * k2 + 2, :],
                        rhs=x8_s[:, 2 * k2:2 * k2 + 2, n * 512:(n + 1) * 512],
                        start=(k2 == 0),
                        stop=(k2 == KO_C // 2 - 1),
                        perf_mode=DR,
                    )
                dest = QT if m < 4 else KT
                # (acc * 1/WSCALE) + bias -> bf16, on DVE (keeps ACT for exp)
                nc.vector.scalar_tensor_tensor(
                    out=dest[:, m % 4, n * 512:(n + 1) * 512],
                    in0=acc[:],
                    scalar=1.0 / WSCALE,
                    in1=bqk_s[:, m:m + 1].to_broadcast((P, 512)),
                    op0=mybir.AluOpType.mult,
                    op1=mybir.AluOpType.add,
                )

            # y^T aliases the first 4 ko-chunks of xbf (x cols there are dead
            # once the V projection for t-chunks 0..3 is done; QK proj reads
            # the separate x8 copy, so no race even for qc=0)
            YT = xbf_s[:, 0:KO_G, :]

            # ---- V-projection / output-projection emitters ----
            def emit_v(t):
                acc = pp.tile([P, GC], f32, tag="pp")
                for ko in range(KO_C):
                    nc.tensor.matmul(
                        acc[:],
                        lhsT=xbf_s[:, ko, t * P:(t + 1) * P],
                        rhs=wv_s[:, ko, :],
                        start=(ko == 0),
                        stop=(ko == KO_C - 1),
                    )
                nc.vector.tensor_tensor(
                    vaug[:, t, :, 0:64],
                    acc[:].rearrange("p (h d) -> p h d", h=HG),
                    bv_s[:],
                    mybir.AluOpType.add,
                )

            def emit_c(t, n):
                opsum = pp.tile([P, 512], f32, tag="pp")
                for ko in range(KO_G):
                    nc.tensor.matmul(
                        opsum[:],
                        lhsT=YT[:, ko, t * P:(t + 1) * P],
                        rhs=wp_s[:, ko, n * 512:(n + 1) * 512],
                        start=(ko == 0),
                        stop=(ko == KO_G - 1),
                    )
                osb = ppool.tile([P, 512], f32, tag="osb")
                nc.vector.tensor_copy(out=osb[:], in_=opsum[:])
                nc.sync.dma_start(
                    out=out[t * P:(t + 1) * P, n * 512:(n + 1) * 512],
                    in_=osb[:],
                )

            # ---- attention for one (q-chunk, head-pair) ----
            # per-kc S PSUM tiles (double-buffered so exp pipelines with the
            # PE); exp writes into kc-pair-shaped SBUF pt tiles (slot s) so a
            # DoubleRow AV can later consume [P, 2, h, q] slices directly.
            def emit_b(qc, hp):
                    nkc = 4 * (qc + 1)
                    ype = py.tile([P, 512], f32, tag="py")
                    ypo = py.tile([P, 512], f32, tag="py")
                    pt = None
                    for kc in range(nkc):
                        s = kc % 2
                        if s == 0:
                            pt = ppool.tile([P, 2, 2, 512], f32r, tag="pt")
                        j = kc - 4 * qc
                        qo = max(j, 0) * P        # valid-q offset in this chunk
                        w = 512 - qo
                        spsum = ps.tile([P, 2, 512], f32, tag="ps")
                        for odd in (0, 1):
                            po = odd * 64
                            nc.tensor.matmul(
                                spsum[:, odd, 0:w],
                                lhsT=KT[po:po + 64, hp, kc * P:(kc + 1) * P],
                                rhs=QT[po:po + 64, hp,
                                       qc * 512 + qo:(qc + 1) * 512],
                                start=True,
                                stop=True,
                            )
                        nc.scalar.activation(
                            pt[:, s, :, 0:w], spsum[:, :, 0:w],
                            mybir.ActivationFunctionType.Exp, scale=0.125,
                        )
                        if j >= 0:
                            nc.vector.tensor_tensor(
                                pt[:, s, :, 0:P], pt[:, s, :, 0:P],
                                tri[:, None, :].to_broadcast((P, 2, P)),
                                mybir.AluOpType.mult,
                            )
                        for odd, yp in ((0, ype), (1, ypo)):
                            nc.tensor.matmul(
                                yp[0:65, qo:512],
                                lhsT=vaug[:, kc, 2 * hp + odd, :],
                                rhs=pt[:, s, odd, 0:w],
                                start=(kc == 0),
                                stop=(kc == nkc - 1),
                            )
                    for odd, yp in ((0, ype), (1, ypo)):
                        po = odd * 64
                        # copy the PSUM out fast so the bank frees for the
                        # next head-pair; normalize from the SBUF copy
                        sum_sb = ppool.tile([1, 512], f32, tag="osb")
                        ycop = spool.tile([64, 512], f32, tag="ycop")
                        nc.vector.tensor_copy(out=sum_sb[:], in_=yp[64:65, :])
                        nc.vector.tensor_copy(out=ycop[:], in_=yp[0:64, :])
                        srep = spool.tile([64, 512], f32, tag="srep")
                        nc.gpsimd.partition_broadcast(srep[:], sum_sb[:])
                        nc.vector.reciprocal_approx_fast(out=srep[:], in_=srep[:])
                        yslice = YT[po:po + 64, hp, qc * 512:(qc + 1) * 512]
                        if odd == 0:
                            nc.vector.tensor_tensor(
                                yslice, ycop[:], srep[:], mybir.AluOpType.mult
                            )
                        else:
                            # DVE lanes can't shift partitions; stage at 0..63
                            # and DMA to partitions 64..127
                            yt_tmp = ppool.tile([64, 512], bf16, tag="ytmp")
                            nc.vector.tensor_tensor(
                                yt_tmp[:], ycop[:], srep[:], mybir.AluOpType.mult
                            )
                            nc.sync.dma_start(out=yslice, in_=yt_tmp[:])

            # ---- schedule ----
            # n-major QK sweep (each group is only 4 DoubleRow matmuls), with
            # V projections and qc=0 attention interleaved as data lands.
            for n in range(NQC):
                for hp in range(4):
                    emit_qk_group(hp, n)       # Q chunk hp, T-slice n
                    emit_qk_group(4 + hp, n)   # K chunk hp, T-slice n
                    if n == 0 and hp == 1:
                        for t in range(4):
                            emit_v(t)
                    if n == 0 and hp >= 2:
                        emit_b(0, hp - 2)      # needs QT/KT pair hp-2 @ n=0
                    if n == 1:
                        if hp == 0:
                            emit_b(0, 2)
                        elif hp == 1:
                            emit_b(0, 3)
                        else:
                            emit_v(2 + hp)     # t = 4, 5
                    if n == 2 and hp < 2:
                        emit_v(6 + hp)         # t = 6, 7
                if n == 2:
                    break
            # remaining QK groups (n=3) interleave with qc=1 attention
            for qc in range(1, NQC):
                for hp in range(HG // 2):
                    if qc == 1:
                        emit_qk_group(hp, 3)
                        emit_qk_group(4 + hp, 3)
                    if qc < NQC - 1:
                        emit_v(4 * (qc + 1) + hp)
                    t = (qc - 1) * 4 + hp
                    emit_c(t, 0)
                    emit_b(qc, hp)
                    emit_c(t, 1)
            # trailing output projection for the last q-chunk
            for t in range(12, 16):
                emit_c(t, 0)
                emit_c(t, 1)
    nc.finalize()
    return nc


def _get_nc():
    if "nc" not in _cache:
        _cache["nc"] = _build()
    return _cache["nc"]


def _prep_inputs(x, W_attn, b_attn, W_proj):
    import ml_dtypes

    bfloat16 = ml_dtypes.bfloat16
    f8 = ml_dtypes.float8_e4m3
    x = np.ascontiguousarray(np.asarray(x, np.float32))
    W_attn = np.asarray(W_attn, np.float32)
    b_attn = np.asarray(b_attn, np.float32)
    W_proj = np.asarray(W_proj, np.float32)
    mask = (np.arange(P)[:, None] <= np.arange(P)[None, :]).astype(np.float32)
    in_maps = []
    for b in range(B):
        xTb = np.ascontiguousarray(x[b].T.reshape(KO_C, P, T).transpose(1, 0, 2))
        x8b = np.clip(xTb, -240, 240).astype(f8)
        xbfb = xTb.astype(bfloat16)
        for g in range(G):
            qs, ks, vs = g * GC, C + g * GC, 2 * C + g * GC
            w2 = np.concatenate([W_attn[:, qs:qs + GC], W_attn[:, ks:ks + GC]], 1)
            in_maps.append({
                "x8": x8b,
                "xbf": xbfb,
                "wqk": np.ascontiguousarray(
                    (w2 * WSCALE).reshape(KO_C, P, 2 * GC // P, P)
                    .transpose(2, 1, 0, 3)).astype(f8),
                "bqk": np.ascontiguousarray(
                    np.concatenate([b_attn[qs:qs + GC], b_attn[ks:ks + GC]])
                    .reshape(2 * GC // P, P).T),
                "wv": np.ascontiguousarray(
                    W_attn[:, vs:vs + GC].reshape(KO_C, P, GC)
                    .transpose(1, 0, 2)).astype(bfloat16),
                "bv": np.ascontiguousarray(
                    np.broadcast_to(b_attn[vs:vs + GC], (P, GC))),
                "wp": np.ascontiguousarray(
                    W_proj[g * GC:(g + 1) * GC, :].reshape(KO_G, P, C)
                    .transpose(1, 0, 2)).astype(bfloat16),
                "mask": mask,
            })
    return in_maps


def _run(inputs, trace=False):
    from concourse.bass_utils import run_bass_kernel_spmd

    nc = _get_nc()
    in_maps = _prep_inputs(
        inputs["x"], inputs["W_attn"], inputs["b_attn"], inputs["W_proj"]
    )
    res = run_bass_kernel_spmd(nc, in_maps, list(range(B * G)), trace=trace)
    b_proj = np.asarray(inputs["b_proj"], np.float32)
    outs = [
        res.results[2 * b]["out"] + res.results[2 * b + 1]["out"] + b_proj
        for b in range(B)
    ]
    return np.stack(outs).astype(np.float32), res


def kernel(**inputs):
    return _run(inputs, trace=False)[0]


if __name__ == "__main__":
    rng = np.random.default_rng(0)
    ins = {
        "x": rng.standard_normal((B, T, C), np.float32),
        "W_attn": rng.uniform(-0.03, 0.03, (C, 3 * C)).astype(np.float32),
        "b_attn": rng.uniform(-0.03, 0.03, (3 * C,)).astype(np.float32),
        "W_proj": rng.uniform(-0.03, 0.03, (C, C)).astype(np.float32),
        "b_proj": rng.uniform(-0.03, 0.03, (C,)).astype(np.float32),
    }
    out = kernel(**ins)
    print("ran, out shape", out.shape)


# revision 20
# speedup vs baseline: 1.4309x; 1.4309x over previous
"""Causal self-attention (B=4, T=2048, C=1024, H=16, HD=64) on 8 trn2 cores.

Sharding: core = (batch b, head-group g), g in {0,1} covering 8 heads each.
Each core: QKV projection for its 8 heads, causal attention, partial output
projection y_g @ W_proj[g*512:(g+1)*512]. Host sums the two partials and adds
the output bias (which absorbs b_v: softmax weights sum to 1, so the V bias
passes through attention exactly and b_eff = b_proj + b_v @ W_proj).

Quantization recipe (max rel err vs f32 reference ~8e-3, gate 2e-2):
  - QK projection: fp8e4 x and W (W pre-scaled x16 on host to stay in fp8
    normals; un-scaled in the PSUM->SBUF copy), DoubleRow matmuls: the HW
    streams 1 col/cycle regardless, but DoubleRow doubles the contraction
    per pass (256), halving instruction count. Q/K stored bf16.
  - S = K^T Q in bf16 (64-deep row-group pairs run concurrently on the PE).
  - P = exp(S/8) in bf16 (ACT); causal masking = lower-triangle multiply on
    the bf16 tile (DVE, 2x mode) for diagonal blocks.
  - AV in bf16 (same col count as fp8-DoubleRow+residual would need, with
    none of the fp8 error). The ones column of V_aug makes PSUM row 64 the
    softmax denominator.
  - y normalized via gpsimd partition-broadcast + fast reciprocal, stored
    bf16; output projection in bf16.

Scheduling: emit_b is software-pipelined at kc granularity (S of kc+1 is
emitted before the AV that waits on exp(kc), so the PE never head-of-line
blocks on the ACT engine); V projections, output projections and the n=3 QK
groups fill PE slack during ACT-bound attention stretches; x streams in
T-major chunks; QK PSUM->SBUF copies run on ACT (idle during phase A).
"""

import numpy as np

B, T, C, H, HD = 4, 2048, 1024, 16, 64
G = 2              # head groups (tensor parallel)
HG = H // G        # 8 heads per group
GC = HG * HD       # 512 group channels
P = 128
NQC = T // 512     # 4 q-chunks of 512
NKC = T // P       # 16 k-chunks of 128
KO_C = C // P      # 8 contraction chunks for C=1024
KO_G = GC // P     # 4 contraction chunks for GC=512
WSCALE = 16.0      # host-side premultiplier on W_attn qk columns (fp8 normals)

_cache = {}


def _build():
    import concourse.bass as bass
    import concourse.tile as tile
    from concourse import bacc, mybir

    f32 = mybir.dt.float32
    f32r = mybir.dt.float32r
    bf16 = mybir.dt.bfloat16
    fp8 = mybir.dt.float8e4
    DR = mybir.MatmulPerfMode.DoubleRow

    nc = bacc.Bacc(name="csa")
    x8 = nc.declare_dram_parameter("x8", [P, KO_C, T], fp8, isOutput=False)
    xbf = nc.declare_dram_parameter("xbf", [P, KO_C, T], bf16, isOutput=False)
    wqk = nc.declare_dram_parameter("wqk", [2 * GC // P, P, KO_C, P], fp8, isOutput=False)
    bqk = nc.declare_dram_parameter("bqk", [P, 2 * GC // P], f32, isOutput=False)
    wv = nc.declare_dram_parameter("wv", [P, KO_C, GC], bf16, isOutput=False)
    wp = nc.declare_dram_parameter("wp", [P, KO_G, C], bf16, isOutput=False)
    mask = nc.declare_dram_parameter("mask", [P, P], bf16, isOutput=False)
    out = nc.declare_dram_parameter("out", [T, C], f32, isOutput=True)

    from contextlib import ExitStack

    with tile.TileContext(nc) as tc, ExitStack() as ctx:
            singles = ctx.enter_context(tc.tile_pool(name="singles", bufs=1))
            ppool = ctx.enter_context(tc.tile_pool(name="ppool", bufs=3))
            spool = ctx.enter_context(tc.tile_pool(name="spool", bufs=2))
            pp = ctx.enter_context(tc.tile_pool(name="pp", bufs=2, space="PSUM"))
            ps = ctx.enter_context(tc.tile_pool(name="ps", bufs=2, space="PSUM"))
            py = ctx.enter_context(tc.tile_pool(name="py", bufs=2, space="PSUM"))
            # ---- resident tensors ----
            xbf_s = singles.tile([P, KO_C, T], bf16, tag="xbf")   # V-proj lhsT
            x8_s = singles.tile([P, KO_C, T], fp8, tag="x8")      # QK-proj rhs
            wqk_s = singles.tile([P, 2 * GC // P, KO_C, P], fp8, tag="wqk")

            # ko-chunked full-T transfers: per-partition lines are 2KB (x8) /
            # 4KB (xbf), vs 512B for T-chunked slices (which ran at ~50GB/s)
            d_prev = None
            for _ko in range(KO_C):
                d_prev = nc.sync.dma_start(out=x8_s[:, _ko, :], in_=x8[:, _ko, :])
            for _m in range(2 * GC // P):
                d_wqk = nc.sync.dma_start(out=wqk_s[:, _m], in_=wqk[_m])
                tile.add_dep_helper(d_wqk.ins, d_prev.ins, reason="dma order")
            d_prev = d_wqk
            for _ko in range(KO_C):
                d2 = nc.sync.dma_start(out=xbf_s[:, _ko, :], in_=xbf[:, _ko, :])
                tile.add_dep_helper(d2.ins, d_prev.ins, reason="dma order")
            d_prev = d2
            wv_s = singles.tile([P, KO_C, GC], bf16, tag="wv")
            for _ko in range(KO_C):
                d2 = nc.sync.dma_start(out=wv_s[:, _ko, :], in_=wv[:, _ko, :])
                tile.add_dep_helper(d2.ins, d_prev.ins, reason="dma order")
            d_prev = d2
            wp_s = singles.tile([P, KO_G, C], bf16, tag="wp")
            for _ko in range(KO_G):
                _d = nc.sync.dma_start(out=wp_s[:, _ko, :], in_=wp[:, _ko, :])
                tile.add_dep_helper(_d.ins, d_prev.ins, reason="dma order")

            QT = singles.tile([P, HG // 2, T], bf16, tag="QT")
            KT = singles.tile([P, HG // 2, T], bf16, tag="KT")
            # V augmented bf16: cols 0..63 = V, col 64 = ones (softmax
            # denominator)
            vaug = singles.tile([P, NKC, HG, 65], bf16, tag="vaug")
            ones_sb = singles.tile([P, 1], f32, tag="ones_sb")
            nc.vector.memset(ones_sb[:], 1.0)
            nc.vector.tensor_copy(
                out=vaug[:, :, :, 64:65],
                in_=ones_sb[:, :, None, None].to_broadcast((P, NKC, HG, 1)),
            )
            tri = singles.tile([P, P], bf16, tag="tri")
            nc.sync.dma_start(out=tri[:], in_=mask[:])
            bqk_s = singles.tile([P, 2 * GC // P], f32, tag="bqk")
            nc.sync.dma_start(out=bqk_s[:], in_=bqk[:])

            # ---- QK projection: fp8 DoubleRow, contraction 256/instr ----
            def emit_qk_group(m, n):
                acc = pp.tile([P, 512], f32, tag="pp")
                for k2 in range(KO_C // 2):
                    nc.tensor.matmul(
                        acc[:],
                        lhsT=wqk_s[:, m, 2 * k2:2 * k2 + 2, :],
                        rhs=x8_s[:, 2 * k2:2 * k2 + 2, n * 512:(n + 1) * 512],
                        start=(k2 == 0),
                        stop=(k2 == KO_C // 2 - 1),
                        perf_mode=DR,
                    )
                dest = QT if m < 4 else KT
                # (acc/WSCALE) + bias -> bf16, on ACT (idle in phase A)
                nc.scalar.activation(
                    dest[:, m % 4, n * 512:(n + 1) * 512], acc[:],
                    mybir.ActivationFunctionType.Identity,
                    bias=bqk_s[:, m:m + 1], scale=1.0 / WSCALE,
                )

            # y^T aliases the first 4 ko-chunks of xbf (x cols there are dead
            # once the V projection for the matching t-chunks is done; QK proj
            # reads the separate x8 copy, so no race even for qc=0)
            YT = xbf_s[:, 0:KO_G, :]

            # ---- V-projection / output-projection emitters ----
            def emit_v(t):
                acc = pp.tile([P, GC], f32, tag="pp")
                for ko in range(KO_C):
                    nc.tensor.matmul(
                        acc[:],
                        lhsT=xbf_s[:, ko, t * P:(t + 1) * P],
                        rhs=wv_s[:, ko, :],
                        start=(ko == 0),
                        stop=(ko == KO_C - 1),
                    )
                nc.vector.tensor_copy(
                    out=vaug[:, t, :, 0:64],
                    in_=acc[:].rearrange("p (h d) -> p h d", h=HG),
                )

            def emit_c(t, n):
                opsum = pp.tile([P, 512], f32, tag="pp")
                for ko in range(KO_G):
                    nc.tensor.matmul(
                        opsum[:],
                        lhsT=YT[:, ko, t * P:(t + 1) * P],
                        rhs=wp_s[:, ko, n * 512:(n + 1) * 512],
                        start=(ko == 0),
                        stop=(ko == KO_G - 1),
                    )
                osb = ppool.tile([P, 512], f32, tag="osb")
                nc.vector.tensor_copy(out=osb[:], in_=opsum[:])
                nc.sync.dma_start(
                    out=out[t * P:(t + 1) * P, n * 512:(n + 1) * 512],
                    in_=osb[:],
                )

            # ---- attention for one (q-chunk, head-pair) ----
            # software-pipelined: S(kc+1) is emitted before the AV of kc, so
            # when the AV waits on exp the PE keeps streaming S.
            def emit_b(qc, hp):
                    nkc = 4 * (qc + 1)
                    qo_of = [max(kc - 4 * qc, 0) * P for kc in range(nkc)]
                    ype = py.tile([P, 512], f32, tag="py")
                    ypo = py.tile([P, 512], f32, tag="py")
                    spsums = {}
                    pts = {}

                    def emit_s(kc):
                        qo = qo_of[kc]
                        w = 512 - qo
                        spsum = ps.tile([P, 2, 512], f32, tag="ps")
                        spsums[kc] = spsum
                        for odd in (0, 1):
                            po = odd * 64
                            nc.tensor.matmul(
                                spsum[:, odd, 0:w],
                                lhsT=KT[po:po + 64, hp, kc * P:(kc + 1) * P],
                                rhs=QT[po:po + 64, hp,
                                       qc * 512 + qo:(qc + 1) * 512],
                                start=True,
                                stop=True,
                            )

                    def emit_exp(kc):
                        qo = qo_of[kc]
                        w = 512 - qo
                        pt = ppool.tile([P, 2, 512], bf16, tag="pt")
                        pts[kc] = pt
                        nc.scalar.activation(
                            pt[:, :, 0:w], spsums.pop(kc)[:, :, 0:w],
                            mybir.ActivationFunctionType.Exp, scale=0.125,
                        )
                        if kc >= 4 * qc:
                            nc.vector.tensor_tensor(
                                pt[:, :, 0:P], pt[:, :, 0:P],
                                tri[:, None, :].to_broadcast((P, 2, P)),
                                mybir.AluOpType.mult,
                            )

                    def emit_av(kc):
                        pt = pts.pop(kc)
                        qo = qo_of[kc]
                        w = 512 - qo
                        for odd, yp in ((0, ype), (1, ypo)):
                            nc.tensor.matmul(
                                yp[0:65, qo:512],
                                lhsT=vaug[:, kc, 2 * hp + odd, :],
                                rhs=pt[:, odd, 0:w],
                                start=(kc == 0),
                                stop=(kc == nkc - 1),
                            )

                    emit_s(0)
                    for kc in range(nkc):
                        emit_exp(kc)
                        if kc + 1 < nkc:
                            emit_s(kc + 1)
                        emit_av(kc)
                    for odd, yp in ((0, ype), (1, ypo)):
                        po = odd * 64
                        # copy the PSUM out fast so the bank frees for the
                        # next head-pair; normalize from the SBUF copy
                        sum_sb = ppool.tile([1, 512], f32, tag="osb")
                        ycop = spool.tile([64, 512], f32, tag="ycop")
                        nc.vector.tensor_copy(out=sum_sb[:], in_=yp[64:65, :])
                        nc.vector.tensor_copy(out=ycop[:], in_=yp[0:64, :])
                        srep = spool.tile([64, 512], f32, tag="srep")
                        nc.gpsimd.partition_broadcast(srep[:], sum_sb[:])
                        nc.vector.reciprocal_approx_fast(out=srep[:], in_=srep[:])
                        yslice = YT[po:po + 64, hp, qc * 512:(qc + 1) * 512]
                        if odd == 0:
                            nc.vector.tensor_tensor(
                                yslice, ycop[:], srep[:], mybir.AluOpType.mult
                            )
                        else:
                            # DVE lanes can't shift partitions; stage at 0..63
                            # and DMA to partitions 64..127
                            yt_tmp = ppool.tile([64, 512], bf16, tag="ytmp")
                            nc.vector.tensor_tensor(
                                yt_tmp[:], ycop[:], srep[:], mybir.AluOpType.mult
                            )
                            nc.sync.dma_start(out=yslice, in_=yt_tmp[:])

            # ---- schedule ----
            # n-major QK sweep (each group is only 4 DoubleRow matmuls), with
            # V projections and qc=0 attention interleaved as data lands.
            for n in range(3):
                for hp in range(4):
                    emit_qk_group(hp, n)       # Q chunk hp, T-slice n
                    emit_qk_group(4 + hp, n)   # K chunk hp, T-slice n
                    if n == 0 and hp == 1:
                        for t in range(4):
                            emit_v(t)
                    if n == 0 and hp >= 2:
                        emit_b(0, hp - 2)      # needs QT/KT pair hp-2 @ n=0
                    if n == 1:
                        if hp == 0:
                            emit_b(0, 2)
                        elif hp == 1:
                            emit_b(0, 3)
                        else:
                            emit_v(2 + hp)     # t = 4, 5
                    if n == 2 and hp < 2:
                        emit_v(6 + hp)         # t = 6, 7
            # remaining QK groups (n=3) interleave with qc=1 attention
            for qc in range(1, NQC):
                for hp in range(HG // 2):
                    if qc == 1:
                        emit_qk_group(hp, 3)
                        emit_qk_group(4 + hp, 3)
                    if qc < NQC - 1:
                        emit_v(4 * (qc + 1) + hp)
                    t = (qc - 1) * 4 + hp
                    emit_c(t, 0)
                    emit_b(qc, hp)
                    emit_c(t, 1)
            # trailing output projection for the last q-chunk
            for t in range(12, 16):
                emit_c(t, 0)
                emit_c(t, 1)
    nc.finalize()
    return nc


def _get_nc():
    if "nc" not in _cache:
        _cache["nc"] = _build()
    return _cache["nc"]


def _prep_inputs(x, W_attn, b_attn, W_proj):
    import ml_dtypes

    bfloat16 = ml_dtypes.bfloat16
    f8 = ml_dtypes.float8_e4m3
    x = np.ascontiguousarray(np.asarray(x, np.float32))
    W_attn = np.asarray(W_attn, np.float32)
    b_attn = np.asarray(b_attn, np.float32)
    W_proj = np.asarray(W_proj, np.float32)
    mask = (np.arange(P)[:, None] <= np.arange(P)[None, :]).astype(np.float32)
    in_maps = []
    for b in range(B):
        xTb = np.ascontiguousarray(x[b].T.reshape(KO_C, P, T).transpose(1, 0, 2))
        x8b = np.clip(xTb, -240, 240).astype(f8)
        xbfb = xTb.astype(bfloat16)
        for g in range(G):
            qs, ks, vs = g * GC, C + g * GC, 2 * C + g * GC
            w2 = np.concatenate([W_attn[:, qs:qs + GC], W_attn[:, ks:ks + GC]], 1)
            in_maps.append({
                "x8": x8b,
                "xbf": xbfb,
                "wqk": np.ascontiguousarray(
                    (w2 * WSCALE).reshape(KO_C, P, 2 * GC // P, P)
                    .transpose(2, 1, 0, 3)).astype(f8),
                "bqk": np.ascontiguousarray(
                    np.concatenate([b_attn[qs:qs + GC], b_attn[ks:ks + GC]])
                    .reshape(2 * GC // P, P).T),
                "wv": np.ascontiguousarray(
                    W_attn[:, vs:vs + GC].reshape(KO_C, P, GC)
                    .transpose(1, 0, 2)).astype(bfloat16),
                "wp": np.ascontiguousarray(
                    W_proj[g * GC:(g + 1) * GC, :].reshape(KO_G, P, C)
                    .transpose(1, 0, 2)).astype(bfloat16),
                "mask": mask.astype(bfloat16),
            })
    return in_maps


def _run(inputs, trace=False):
    from concourse.bass_utils import run_bass_kernel_spmd

    nc = _get_nc()
    in_maps = _prep_inputs(
        inputs["x"], inputs["W_attn"], inputs["b_attn"], inputs["W_proj"]
    )
    res = run_bass_kernel_spmd(nc, in_maps, list(range(B * G)), trace=trace)
    W_proj_f = np.asarray(inputs["W_proj"], np.float32)
    b_attn_f = np.asarray(inputs["b_attn"], np.float32)
    # b_v passes through softmax exactly (weights sum to 1): fold it into the
    # output bias instead of adding it to V in the kernel
    b_eff = (np.asarray(inputs["b_proj"], np.float32)
             + b_attn_f[2 * C:] @ W_proj_f)
    outs = [
        res.results[2 * b]["out"] + res.results[2 * b + 1]["out"] + b_eff
        for b in range(B)
    ]
    return np.stack(outs).astype(np.float32), res


def kernel(**inputs):
    return _run(inputs, trace=False)[0]


if __name__ == "__main__":
    rng = np.random.default_rng(0)
    ins = {
        "x": rng.standard_normal((B, T, C), np.float32),
        "W_attn": rng.uniform(-0.03, 0.03, (C, 3 * C)).astype(np.float32),
        "b_attn": rng.uniform(-0.03, 0.03, (3 * C,)).astype(np.float32),
        "W_proj": rng.uniform(-0.03, 0.03, (C, C)).astype(np.float32),
        "b_proj": rng.uniform(-0.03, 0.03, (C,)).astype(np.float32),
    }
    out = kernel(**ins)
    print("ran, out shape", out.shape)
